# revision 1
# baseline (speedup 1.0000x reference)
"""Trainium2 Bass kernel for GroupedQueryAttention.

Sharding: 8 cores; core c owns KV head g=c and Q heads 4c..4c+3, both batch
elements. Each core computes its [2, 2048, 256] output slice; host concats.

Per-core dataflow (per batch b):
  A) hs [2048, 2048] is loaded row-natural and PE-transposed (is_transpose
     matmul vs identity) into hsT chunks [128 d, 512 s]; projections
     Q^T (2x128 rows), [K^T|V^T] (128 rows) accumulate over the 16 d-tiles.
     1/sqrt(HD) is folded into Wq/bq on the host.
  B) V^T rows are PE-transposed back to natural V [s_k, 64] and a ones
     column is appended -> [V|1] so the PV matmul also produces the softmax
     denominator (row 64 of the output).
  C) Scores are computed transposed, S^T [s_k, s_q]; exp on ACT directly
     PSUM->SBUF (no max subtraction: |scores| < ~6 at this data scale);
     ctxU^T [65, s_q] = [V|1]^T @ expS^T accumulates over s_k tiles in PSUM.
     Small PE transposes bring ctxU^T back to natural [s_q, 65]; DVE does
     1/denominator and the scale-multiply into the output tile.

All matmul operands use float32r (fp32 bits, fast PE path).
"""

import sys
from contextlib import ExitStack

import numpy as np

sys.path.insert(0, "/opt/trn_rl_repo")

import concourse.bass as bass  # noqa: E402
import concourse.bacc as bacc  # noqa: E402
import concourse.tile as tile  # noqa: E402
from concourse import mybir  # noqa: E402
from concourse.bass_utils import run_bass_kernel_spmd  # noqa: E402

B = 2
S = 2048
D = 2048
HD = 64
NCORES = 8
QH = 4           # q heads per core
MCOLS = QH * HD  # 256 output cols per core

MM_DT = mybir.dt.float32r
F32 = mybir.dt.float32
Exp = mybir.ActivationFunctionType.Exp

NDT = 16         # d tiles of 128
NSC = 4          # s chunks of 512 per batch
NKT = 16         # s_k tiles of 128
NSQ = 2          # s_q chunks of 1024


def build_nc():
    nc = bacc.Bacc("TRN2", target_bir_lowering=False, debug=False)

    hs_d = nc.dram_tensor("hs", [B, S, D], MM_DT, kind="ExternalInput")
    wq_d = nc.dram_tensor("wq", [D, MCOLS], MM_DT, kind="ExternalInput")
    wkv_d = nc.dram_tensor("wkv", [D, 128], MM_DT, kind="ExternalInput")
    bq_d = nc.dram_tensor("bq", [128, 2], F32, kind="ExternalInput")
    bkv_d = nc.dram_tensor("bkv", [128, 1], F32, kind="ExternalInput")
    id_d = nc.dram_tensor("ident", [128, 128], MM_DT, kind="ExternalInput")
    out_d = nc.dram_tensor("out", [B, S, MCOLS], F32, kind="ExternalOutput")

    with tile.TileContext(nc) as tc, ExitStack() as ctx:
        const = ctx.enter_context(tc.tile_pool(name="const", bufs=1))
        wqp = ctx.enter_context(tc.tile_pool(name="wqp", bufs=NDT))
        wkvp = ctx.enter_context(tc.tile_pool(name="wkvp", bufs=NDT))
        natp = ctx.enter_context(tc.tile_pool(name="natp", bufs=2))
        hstp = ctx.enter_context(tc.tile_pool(name="hstp", bufs=NDT + 2))
        qtp = ctx.enter_context(tc.tile_pool(name="qtp", bufs=4))
        kvp = ctx.enter_context(tc.tile_pool(name="kvp", bufs=2))
        kthp = ctx.enter_context(tc.tile_pool(name="kthp", bufs=2))
        v1p = ctx.enter_context(tc.tile_pool(name="v1p", bufs=2 * NKT))
        expp = ctx.enter_context(tc.tile_pool(name="expp", bufs=3))
        up = ctx.enter_context(tc.tile_pool(name="up", bufs=2))
        recp = ctx.enter_context(tc.tile_pool(name="recp", bufs=4))
        outp = ctx.enter_context(tc.tile_pool(name="outp", bufs=16))
        psp = ctx.enter_context(tc.tile_pool(name="psp", bufs=3, space="PSUM"))
        pvp = ctx.enter_context(tc.tile_pool(name="pvp", bufs=1, space="PSUM"))

        ident = const.tile([128, 128], MM_DT, tag="ident")
        nc.sync.dma_start(out=ident[:], in_=id_d[:])
        bq_sb = const.tile([128, 2], F32, tag="bq")
        nc.sync.dma_start(out=bq_sb[:], in_=bq_d[:])
        bkv_sb = const.tile([128, 1], F32, tag="bkv")
        nc.sync.dma_start(out=bkv_sb[:], in_=bkv_d[:])
        zb = const.tile([128, 1], F32, tag="zb")
        nc.vector.memset(zb[:], 0.0)
        ones_sb = const.tile([128, 1], F32, tag="ones")
        nc.vector.memset(ones_sb[:], 1.0)
        zero64 = const.tile([128, 64], F32, tag="zero64")
        nc.vector.memset(zero64[:], 0.0)

        wq_sb = []
        wkv_sb = []
        for dt_ in range(NDT):
            w = wqp.tile([128, MCOLS], MM_DT, tag="wq")
            nc.sync.dma_start(out=w[:], in_=wq_d[dt_ * 128:(dt_ + 1) * 128, :])
            wq_sb.append(w)
            w2 = wkvp.tile([128, 128], MM_DT, tag="wkv")
            nc.sync.dma_start(out=w2[:], in_=wkv_d[dt_ * 128:(dt_ + 1) * 128, :])
            wkv_sb.append(w2)

        for b in range(B):
            # ---- Phase A: transpose hs + projections ----
            qT = [qtp.tile([128, S], MM_DT, tag="qt", name=f"qT{b}_{i}") for i in range(2)]
            kvT = kvp.tile([128, S], MM_DT, tag="kv")
            for sc in range(NSC):
                hsT = [hstp.tile([128, 512], MM_DT, tag="hst", name=f"hsT{b}_{sc}_{i}") for i in range(NDT)]
                for st in range(4):
                    r0 = sc * 512 + st * 128
                    nat = natp.tile([128, D], MM_DT, tag="nat")
                    nc.sync.dma_start(out=nat[:], in_=hs_d[b, r0:r0 + 128, :])
                    for dt_ in range(NDT):
                        pst = psp.tile([128, 128], MM_DT, tag="ps")
                        nc.tensor.transpose(
                            pst[:], nat[:, dt_ * 128:(dt_ + 1) * 128], ident[:]
                        )
                        nc.vector.tensor_copy(
                            hsT[dt_][:, st * 128:(st + 1) * 128], pst[:]
                        )
                c0, c1 = sc * 512, (sc + 1) * 512
                for qc in range(2):
                    ps = psp.tile([128, 512], F32, tag="ps")
                    for dt_ in range(NDT):
                        nc.tensor.matmul(
                            ps[:], wq_sb[dt_][:, qc * 128:(qc + 1) * 128],
                            hsT[dt_][:], start=(dt_ == 0), stop=(dt_ == NDT - 1),
                        )
                    nc.vector.tensor_scalar_add(
                        qT[qc][:, c0:c1], ps[:], bq_sb[:, qc:qc + 1]
                    )
                ps = psp.tile([128, 512], F32, tag="ps")
                for dt_ in range(NDT):
                    nc.tensor.matmul(
                        ps[:], wkv_sb[dt_][:], hsT[dt_][:],
                        start=(dt_ == 0), stop=(dt_ == NDT - 1),
                    )
                nc.vector.tensor_scalar_add(kvT[:, c0:c1], ps[:], bkv_sb[:])

            kth = kthp.tile([128, S], MM_DT, tag="kth")
            nc.sync.dma_start(out=kth[64:128, :], in_=kvT[0:64, :])

            # ---- Phase B: V natural + ones column ----
            v1 = []
            for kt in range(NKT):
                pst = psp.tile([128, 64], MM_DT, tag="ps")
                nc.tensor.transpose(
                    pst[:], kvT[64:128, kt * 128:(kt + 1) * 128],
                    ident[64:128, 64:128],
                )
                v = v1p.tile([128, 128], MM_DT, tag="v1")
                nc.vector.tensor_copy(v[:, 0:64], pst[:])
                nc.vector.tensor_copy(v[:, 64:128], zero64[:])
                nc.vector.tensor_copy(v[:, 64:65], ones_sb[:])
                v1.append(v)

            # ---- Phase C: attention ----
            outt = [outp.tile([128, MCOLS], F32, tag="out", name=f"outt{b}_{i}") for i in range(16)]
            for h in range(QH):
                qrow = (h % 2) * 64
                qt = qT[h // 2]
                for sq in range(NSQ):
                    q0 = sq * 1024
                    pv = pvp.tile([128, 1024], F32, tag="pv")
                    for kt in range(NKT):
                        pss = psp.tile([128, 1024], F32, tag="ps")
                        kmat = kvT if qrow == 0 else kth
                        for qc in range(2):
                            nc.tensor.matmul(
                                pss[:, qc * 512:(qc + 1) * 512],
                                kmat[qrow:qrow + 64, kt * 128:(kt + 1) * 128],
                                qt[qrow:qrow + 64,
                                   q0 + qc * 512:q0 + (qc + 1) * 512],
                                start=True, stop=True,
                            )
                        ex = expp.tile([128, 1024], MM_DT, tag="exp")
                        nc.scalar.activation(ex[:], pss[:], Exp, bias=zb[:])
                        for qc in range(2):
                            nc.tensor.matmul(
                                pv[:, qc * 512:(qc + 1) * 512],
                                v1[kt][:], ex[:, qc * 512:(qc + 1) * 512],
                                start=(kt == 0), stop=(kt == NKT - 1),
                            )
                    u = up.tile([128, 1024], MM_DT, tag="u")
                    nc.vector.tensor_copy(u[:], pv[:])
                    for tb in range(8):
                        pst = psp.tile([128, 128], MM_DT, tag="ps")
                        nc.tensor.transpose(
                            pst[:], u[:, tb * 128:(tb + 1) * 128],
                            ident[:],
                        )
                        rec = recp.tile([128, 1], F32, tag="rec")
                        nc.vector.reciprocal(rec[:], pst[:, 64:65])
                        st_i = sq * 8 + tb
                        nc.vector.tensor_scalar_mul(
                            outt[st_i][:, h * 64:(h + 1) * 64],
                            pst[:, 0:64], rec[:],
                        )
            for st_i in range(16):
                nc.sync.dma_start(
                    out=out_d[b, st_i * 128:(st_i + 1) * 128, :],
                    in_=outt[st_i][:],
                )

    nc.compile()
    return nc


def make_in_maps(hidden_states, Wq, bq, Wk, bk, Wv, bv):
    hs = np.ascontiguousarray(np.asarray(hidden_states, dtype=np.float32))
    Wq = np.asarray(Wq, dtype=np.float32)
    bq = np.asarray(bq, dtype=np.float32)
    Wk = np.asarray(Wk, dtype=np.float32)
    bk = np.asarray(bk, dtype=np.float32)
    Wv = np.asarray(Wv, dtype=np.float32)
    bv = np.asarray(bv, dtype=np.float32)
    sc = 1.0 / np.sqrt(np.float32(HD))
    ident = np.eye(128, dtype=np.float32)
    in_maps = []
    for c in range(NCORES):
        qs = slice(c * MCOLS, (c + 1) * MCOLS)
        ks = slice(c * HD, (c + 1) * HD)
        bq_c = (bq[qs] * sc).reshape(2, 128).T
        in_maps.append({
            "hs": hs,
            "wq": np.ascontiguousarray(Wq[:, qs] * sc),
            "wkv": np.ascontiguousarray(
                np.concatenate([Wk[:, ks], Wv[:, ks]], axis=1)),
            "bq": np.ascontiguousarray(bq_c),
            "bkv": np.concatenate([bk[ks], bv[ks]]).reshape(128, 1),
            "ident": ident,
        })
    return in_maps


_NC_CACHE = {}


def get_nc():
    if "nc" not in _NC_CACHE:
        _NC_CACHE["nc"] = build_nc()
    return _NC_CACHE["nc"]


def kernel(hidden_states, Wq, bq, Wk, bk, Wv, bv):
    nc = get_nc()
    in_maps = make_in_maps(hidden_states, Wq, bq, Wk, bk, Wv, bv)
    res = run_bass_kernel_spmd(nc, in_maps, list(range(NCORES)))
    outs = [np.asarray(r["out"], dtype=np.float32) for r in res.results]
    return np.concatenate(outs, axis=-1)



# revision 6
# speedup vs baseline: 1.7265x; 1.7265x over previous
"""Trainium2 Bass kernel for GroupedQueryAttention (v2).

Sharding: 8 cores; core c owns KV head g=c and Q heads 4c..4c+3, both batch
elements. Each core computes its [2, 2048, 256] output slice; host concats.

Key design points vs v1:
  - hs is pre-transposed on the host to [B, D, S] and cast to bf16, so the
    kernel DMAs hs^T tiles directly: no PE transposes / PSUM->SBUF copies
    for the activations, and half the HBM traffic.
  - All matmul operands are bf16 (fp32 PSUM accumulation). bf16 runs at
    1 col/cycle on the PE regardless of output width.
  - PV is computed in the natural orientation: ctx[s_q, d] accumulates
    lhsT=exp(S^T) tile [128k, 128sq], rhs=[V|1] [128k, 65] over k tiles.
    The ones column produces the softmax denominator; DVE does the
    reciprocal-scale into the output tile. No ctx transposes.
  - Phase C is a flat software-pipelined schedule: scores(g) -> exp(g) on
    ACT -> PV(g-1), so the ACT engine (the exp roofline, ~266us) streams
    back-to-back. Batch 1's projections are interleaved into batch 0's
    attention stream via a generator to hide them in PE slack.
"""

import sys
from contextlib import ExitStack

import numpy as np

sys.path.insert(0, "/opt/trn_rl_repo")

import concourse.bass as bass  # noqa: E402
import concourse.bacc as bacc  # noqa: E402
import concourse.tile as tile  # noqa: E402
from concourse import mybir  # noqa: E402
from concourse.bass_utils import run_bass_kernel_spmd  # noqa: E402

B = 2
S = 2048
D = 2048
HD = 64
NCORES = 8
QH = 4           # q heads per core
MCOLS = QH * HD  # 256 output cols per core

BF16 = mybir.dt.bfloat16
F32 = mybir.dt.float32
Exp = mybir.ActivationFunctionType.Exp

NDT = 16         # d tiles of 128
NKT = 16         # s_k tiles of 128
NSQ = 2          # s_q chunks of 1024
NSL = 4          # s slices of 512 for projections

BF16_NP = mybir.dt.np(BF16)


def build_nc():
    nc = bacc.Bacc("TRN2", target_bir_lowering=False, debug=False)

    hst_d = nc.dram_tensor("hst", [B, D, S], BF16, kind="ExternalInput")
    wq_d = nc.dram_tensor("wq", [D, MCOLS], BF16, kind="ExternalInput")
    wkv_d = nc.dram_tensor("wkv", [D, 128], BF16, kind="ExternalInput")
    bq_d = nc.dram_tensor("bq", [128, 2], F32, kind="ExternalInput")
    bkv_d = nc.dram_tensor("bkv", [128, 1], F32, kind="ExternalInput")
    id_d = nc.dram_tensor("ident", [128, 128], BF16, kind="ExternalInput")
    out_d = nc.dram_tensor("out", [B, S, MCOLS], F32, kind="ExternalOutput")

    with tile.TileContext(nc) as tc, ExitStack() as ctx:
        const = ctx.enter_context(tc.tile_pool(name="const", bufs=1))
        hstp = ctx.enter_context(tc.tile_pool(name="hstp", bufs=NDT))
        qtp = ctx.enter_context(tc.tile_pool(name="qtp", bufs=4))
        kvp = ctx.enter_context(tc.tile_pool(name="kvp", bufs=2))
        kthp = ctx.enter_context(tc.tile_pool(name="kthp", bufs=2))
        v1p = ctx.enter_context(tc.tile_pool(name="v1p", bufs=2 * NKT))
        expp = ctx.enter_context(tc.tile_pool(name="expp", bufs=3))
        recp = ctx.enter_context(tc.tile_pool(name="recp", bufs=8))
        outp = ctx.enter_context(tc.tile_pool(name="outp", bufs=32))
        pssp = ctx.enter_context(tc.tile_pool(name="pssp", bufs=2, space="PSUM"))
        ctxp = ctx.enter_context(tc.tile_pool(name="ctxp", bufs=1, space="PSUM"))
        psap = ctx.enter_context(tc.tile_pool(name="psap", bufs=2, space="PSUM"))

        ident = const.tile([128, 128], BF16, tag="ident")
        nc.sync.dma_start(out=ident[:], in_=id_d[:])
        bq_sb = const.tile([128, 2], F32, tag="bq")
        nc.sync.dma_start(out=bq_sb[:], in_=bq_d[:])
        bkv_sb = const.tile([128, 1], F32, tag="bkv")
        nc.sync.dma_start(out=bkv_sb[:], in_=bkv_d[:])
        zb = const.tile([128, 1], F32, tag="zb")
        nc.vector.memset(zb[:], 0.0)

        # Weights: one big DMA each (3-dim AP: partition, d-tile, col).
        wq_sb = const.tile([128, NDT * MCOLS], BF16, tag="wq")
        nc.sync.dma_start(
            out=wq_sb[:].rearrange("p (t c) -> p t c", t=NDT),
            in_=wq_d[:].rearrange("(t p) c -> p t c", p=128),
        )
        wkv_sb = const.tile([128, NDT * 128], BF16, tag="wkv")
        nc.sync.dma_start(
            out=wkv_sb[:].rearrange("p (t c) -> p t c", t=NDT),
            in_=wkv_d[:].rearrange("(t p) c -> p t c", p=128),
        )

        # ---- Phase A (per batch), emitted as generators so later parts can
        # be interleaved into the previous batch's attention stream.
        def proj(st, dst, lhs_col0, lhs_w, bias_ap):
            hsT = st["hsT"]
            for sl in range(NSL):
                c0, c1 = sl * 512, (sl + 1) * 512
                ps = psap.tile([128, 512], F32, tag="psa")
                for dt_ in range(NDT):
                    nc.tensor.matmul(
                        ps[:],
                        lhs_w[:, lhs_col0(dt_):lhs_col0(dt_) + 128],
                        hsT[dt_][:, c0:c1],
                        start=(dt_ == 0), stop=(dt_ == NDT - 1),
                    )
                    yield
                nc.vector.tensor_scalar_add(dst[:, c0:c1], ps[:], bias_ap)

        def phase_a_main(b, st):
            """DMAs, KV projection, kth copy, V tiles, Q block 0."""
            hsT = [hstp.tile([128, S], BF16, tag="hst", name=f"hsT{b}_{i}")
                   for i in range(NDT)]
            st["hsT"] = hsT
            # 2 DMA pieces of 1024 cols per d-tile; piece 0 for all tiles
            # first so slice-0 projections can start earliest.
            for piece in range(2):
                c0, c1 = piece * 1024, (piece + 1) * 1024
                for dt_ in range(NDT):
                    nc.sync.dma_start(
                        out=hsT[dt_][:, c0:c1],
                        in_=hst_d[b, dt_ * 128:(dt_ + 1) * 128, c0:c1],
                    )
            yield
            qT = [qtp.tile([128, S], BF16, tag="qt", name=f"qT{b}_{i}")
                  for i in range(2)]
            kvT = kvp.tile([128, S], BF16, tag="kv")
            st["qT"] = qT
            st["kvT"] = kvT
            # KV first (phase C h0 + v1 need it), then q block 0.
            yield from proj(st, kvT, lambda dt_: dt_ * 128, wkv_sb, bkv_sb[:])
            kth = kthp.tile([128, S], BF16, tag="kth")
            st["kth"] = kth
            nc.sync.dma_start(out=kth[64:128, :], in_=kvT[0:64, :])
            # V natural + ones column
            v1 = []
            for kt in range(NKT):
                pst = psap.tile([128, 64], BF16, tag="psa")
                nc.tensor.transpose(
                    pst[:], kvT[64:128, kt * 128:(kt + 1) * 128],
                    ident[64:128, 64:128],
                )
                v = v1p.tile([128, 65], BF16, tag="v1")
                nc.vector.tensor_copy(v[:, 0:64], pst[:])
                nc.vector.memset(v[:, 64:65], 1.0)
                v1.append(v)
                yield
            st["v1"] = v1
            yield from proj(st, qT[0], lambda dt_: dt_ * 256, wq_sb,
                            bq_sb[:, 0:1])

        def phase_a_q1(b, st):
            yield from proj(st, st["qT"][1], lambda dt_: dt_ * 256 + 128,
                            wq_sb, bq_sb[:, 1:2])

        def run_gen(g, n):
            for _ in range(n):
                if g is None:
                    return None
                try:
                    next(g)
                except StopIteration:
                    return None
            return g

        # ---- Phase C (per batch): flat pipelined schedule over
        # g = (iter, kt) with iter = (h, sq). interleave: generator of the
        # NEXT batch's phase A, advanced a couple of steps per g.
        def phase_c(b, st, interleave):
            qT, kvT, kth, v1 = st["qT"], st["kvT"], st["kth"], st["v1"]
            iters = [(h, sq) for h in range(QH) for sq in range(NSQ)]
            nstep = len(iters) * NKT
            outt = [outp.tile([128, MCOLS], F32, tag="out",
                              name=f"outt{b}_{i}") for i in range(16)]
            prev = None   # (ex_tile, kt, ctx_tile)
            cur_ctx = None

            def scores_exp(g):
                nonlocal cur_ctx
                it, kt = divmod(g, NKT)
                h, sq = iters[it]
                qrow = (h % 2) * 64
                kmat = kvT if qrow == 0 else kth
                qt = qT[h // 2]
                pss = pssp.tile([128, 1024], F32, tag="pss")
                q0 = sq * 1024
                for qc in range(2):
                    nc.tensor.matmul(
                        pss[:, qc * 512:(qc + 1) * 512],
                        kmat[qrow:qrow + 64, kt * 128:(kt + 1) * 128],
                        qt[qrow:qrow + 64, q0 + qc * 512:q0 + (qc + 1) * 512],
                        start=True, stop=True,
                    )
                ex = expp.tile([128, 1024], BF16, tag="exp")
                nc.scalar.activation(ex[:], pss[:], Exp, bias=zb[:])
                if kt == 0:
                    cur_ctx = ctxp.tile([128, 1024], F32, tag="ctx")
                return (ex, kt, cur_ctx)

            def pv(state):
                # start marks the whole 2KB psum bank pending-zero, so only
                # the first block per bank starts the group; the other blocks'
                # first writes land on pending bytes and overwrite correctly.
                ex, kt, ctx_t = state
                for blk in range(8):
                    nc.tensor.matmul(
                        ctx_t[:, blk * 128:blk * 128 + 65],
                        ex[:, blk * 128:(blk + 1) * 128],
                        v1[kt][:],
                        start=(kt == 0 and blk % 4 == 0),
                        stop=(kt == NKT - 1 and blk % 4 == 3),
                    )

            def norm(it, ctx_t):
                h, sq = iters[it]
                for blk in range(8):
                    rec = recp.tile([128, 1], F32, tag="rec")
                    nc.vector.reciprocal(
                        rec[:], ctx_t[:, blk * 128 + 64:blk * 128 + 65])
                    st_i = sq * 8 + blk
                    nc.vector.tensor_scalar_mul(
                        outt[st_i][:, h * 64:(h + 1) * 64],
                        ctx_t[:, blk * 128:blk * 128 + 64], rec[:],
                    )
                if h == QH - 1:
                    for blk in range(8):
                        st_i = sq * 8 + blk
                        nc.sync.dma_start(
                            out=out_d[b, st_i * 128:(st_i + 1) * 128, :],
                            in_=outt[st_i][:],
                        )

            for g in range(nstep):
                state = scores_exp(g)
                if prev is not None:
                    pv(prev)
                    if prev[1] == NKT - 1:
                        norm(g // NKT - 1, prev[2])
                prev = state
                interleave = run_gen(interleave, 2)
            pv(prev)
            norm(len(iters) - 1, prev[2])
            return interleave

        def chain(*gens):
            for g in gens:
                yield from g

        st0, st1 = {}, {}
        for _ in phase_a_main(0, st0):   # emit eagerly: C0 needs it
            pass
        rest = chain(phase_a_q1(0, st0), phase_a_main(1, st1),
                     phase_a_q1(1, st1))
        rest = phase_c(0, st0, rest)
        rest = phase_c(1, st1, rest)
        while rest is not None:          # safety net; normally exhausted
            rest = run_gen(rest, 16)

    nc.compile()
    return nc


def make_in_maps(hidden_states, Wq, bq, Wk, bk, Wv, bv):
    hs = np.asarray(hidden_states, dtype=np.float32)
    hst = np.ascontiguousarray(hs.transpose(0, 2, 1)).astype(BF16_NP)
    Wq = np.asarray(Wq, dtype=np.float32)
    bq = np.asarray(bq, dtype=np.float32)
    Wk = np.asarray(Wk, dtype=np.float32)
    bk = np.asarray(bk, dtype=np.float32)
    Wv = np.asarray(Wv, dtype=np.float32)
    bv = np.asarray(bv, dtype=np.float32)
    sc = 1.0 / np.sqrt(np.float32(HD))
    ident = np.eye(128, dtype=np.float32).astype(BF16_NP)
    in_maps = []
    for c in range(NCORES):
        qs = slice(c * MCOLS, (c + 1) * MCOLS)
        ks = slice(c * HD, (c + 1) * HD)
        bq_c = (bq[qs] * sc).reshape(2, 128).T
        in_maps.append({
            "hst": hst,
            "wq": np.ascontiguousarray(Wq[:, qs] * sc).astype(BF16_NP),
            "wkv": np.ascontiguousarray(
                np.concatenate([Wk[:, ks], Wv[:, ks]], axis=1)).astype(BF16_NP),
            "bq": np.ascontiguousarray(bq_c),
            "bkv": np.concatenate([bk[ks], bv[ks]]).reshape(128, 1),
            "ident": ident,
        })
    return in_maps


_NC_CACHE = {}


def get_nc():
    if "nc" not in _NC_CACHE:
        _NC_CACHE["nc"] = build_nc()
    return _NC_CACHE["nc"]


def kernel(hidden_states, Wq, bq, Wk, bk, Wv, bv):
    nc = get_nc()
    in_maps = make_in_maps(hidden_states, Wq, bq, Wk, bk, Wv, bv)
    res = run_bass_kernel_spmd(nc, in_maps, list(range(NCORES)))
    outs = [np.asarray(r["out"], dtype=np.float32) for r in res.results]
    return np.concatenate(outs, axis=-1)


# revision 12
# speedup vs baseline: 1.7717x; 1.0262x over previous
"""Trainium2 Bass kernel for GroupedQueryAttention (v3).

Sharding: 8 cores; core c owns KV head g=c and Q heads 4c..4c+3, both batch
elements. Each core computes its [2, 2048, 256] output slice; host concats.

Design:
  - hs is pre-transposed on the host to [B, D, S] and cast to bf16, so the
    kernel DMAs hs^T directly: no PE transposes / PSUM->SBUF copies for the
    activations, and half the HBM traffic. One big SBUF tile per batch
    (reused) loaded by 2 wide DMAs (HWDGE issue cost is per-DMA).
  - All matmul operands are bf16 (fp32 PSUM accumulation). bf16 runs at
    1 col/cycle on the PE regardless of output width.
  - PV in natural orientation: ctx[s_q, 0:65] accumulates lhsT=exp(S^T)
    block [128k, 128sq], rhs=[V|1] [128k, 65] over k tiles. The ones column
    yields the softmax denominator; DVE reciprocal-scales into the output
    tile. No ctx transposes. Only the first/last psum block per 2KB bank
    carries start/stop (start marks the whole bank pending-zero).
  - Phase C is a flat software-pipelined schedule over (sq, h, kt):
    scores(g) -> PV(g-1) -> exp(g), so ACT (the exp roofline, ~266us)
    streams back-to-back. Projections for later q blocks and for batch 1
    are interleaved into the attention stream via generators sized to the
    PE slack per step.
"""

import sys
from contextlib import ExitStack

import numpy as np

sys.path.insert(0, "/opt/trn_rl_repo")

import concourse.bass as bass  # noqa: E402
import concourse.bacc as bacc  # noqa: E402
import concourse.tile as tile  # noqa: E402
from concourse import mybir  # noqa: E402
from concourse.bass_utils import run_bass_kernel_spmd  # noqa: E402

B = 2
S = 2048
D = 2048
HD = 64
NCORES = 8
QH = 4           # q heads per core
MCOLS = QH * HD  # 256 output cols per core

BF16 = mybir.dt.bfloat16
F32 = mybir.dt.float32
Exp = mybir.ActivationFunctionType.Exp

NDT = 16         # d tiles of 128
NKT = 16         # s_k tiles of 128
NSQ = 2          # s_q chunks of 1024
NSL = 4          # s slices of 512 for projections

BF16_NP = mybir.dt.np(BF16)


def build_nc():
    nc = bacc.Bacc("TRN2", target_bir_lowering=False, debug=False)

    hst_d = nc.dram_tensor("hst", [B, D, S], BF16, kind="ExternalInput")
    wq_d = nc.dram_tensor("wq", [D, MCOLS], BF16, kind="ExternalInput")
    wkv_d = nc.dram_tensor("wkv", [D, 128], BF16, kind="ExternalInput")
    bq_d = nc.dram_tensor("bq", [128, 2], F32, kind="ExternalInput")
    bkv_d = nc.dram_tensor("bkv", [128, 1], F32, kind="ExternalInput")
    id_d = nc.dram_tensor("ident", [128, 128], BF16, kind="ExternalInput")
    out_d = nc.dram_tensor("out", [B, S, MCOLS], F32, kind="ExternalOutput")

    with tile.TileContext(nc) as tc, ExitStack() as ctx:
        const = ctx.enter_context(tc.tile_pool(name="const", bufs=1))
        hstp = ctx.enter_context(tc.tile_pool(name="hstp", bufs=1))
        qtp = ctx.enter_context(tc.tile_pool(name="qtp", bufs=4))
        kvp = ctx.enter_context(tc.tile_pool(name="kvp", bufs=2))
        kthp = ctx.enter_context(tc.tile_pool(name="kthp", bufs=2))
        v1p = ctx.enter_context(tc.tile_pool(name="v1p", bufs=2 * NKT))
        expp = ctx.enter_context(tc.tile_pool(name="expp", bufs=6))
        recp = ctx.enter_context(tc.tile_pool(name="recp", bufs=8))
        outp = ctx.enter_context(tc.tile_pool(name="outp", bufs=32))
        pssp = ctx.enter_context(tc.tile_pool(name="pssp", bufs=2, space="PSUM"))
        ctxp = ctx.enter_context(tc.tile_pool(name="ctxp", bufs=1, space="PSUM"))
        psap = ctx.enter_context(tc.tile_pool(name="psap", bufs=2, space="PSUM"))

        ident = const.tile([128, 128], BF16, tag="ident")
        bq_sb = const.tile([128, 2], F32, tag="bq")
        bkv_sb = const.tile([128, 1], F32, tag="bkv")
        zb = const.tile([128, 1], F32, tag="zb")
        nc.vector.memset(zb[:], 0.0)
        wq_sb = const.tile([128, NDT * MCOLS], BF16, tag="wq")
        wkv_sb = const.tile([128, NDT * 128], BF16, tag="wkv")

        # DMA issue order matters (one shared HWDGE): small consts, wkv,
        # then batch 0's first hs half, then wq, then the rest.
        nc.sync.dma_start(out=ident[:], in_=id_d[:])
        nc.sync.dma_start(out=bq_sb[:], in_=bq_d[:])
        nc.sync.dma_start(out=bkv_sb[:], in_=bkv_d[:])
        nc.sync.dma_start(
            out=wkv_sb[:].rearrange("p (t c) -> p t c", t=NDT),
            in_=wkv_d[:].rearrange("(t p) c -> p t c", p=128),
        )

        hsT = const.tile([128, NDT * S], BF16, tag="hst")

        def hst_dma(b, half):
            c0, c1 = half * 1024, (half + 1) * 1024
            nc.sync.dma_start(
                out=hsT[:].rearrange("p (t s) -> p t s", t=NDT)[:, :, c0:c1],
                in_=hst_d[b].rearrange("(t p) s -> p t s", p=128)[:, :, c0:c1],
            )

        hst_dma(0, 0)
        nc.sync.dma_start(
            out=wq_sb[:].rearrange("p (t c) -> p t c", t=NDT),
            in_=wq_d[:].rearrange("(t p) c -> p t c", p=128),
        )
        hst_dma(0, 1)

        # ---- Phase A pieces (generators; one yield per PE matmul).
        def proj(dst, lhs_col0, lhs_w, bias_ap, slices):
            for sl in slices:
                c0, c1 = sl * 512, (sl + 1) * 512
                ps = psap.tile([128, 512], F32, tag="psa", name=f"psa_{nc.next_id()}")
                for dt_ in range(NDT):
                    nc.tensor.matmul(
                        ps[:],
                        lhs_w[:, lhs_col0(dt_):lhs_col0(dt_) + 128],
                        hsT[:, dt_ * S + c0:dt_ * S + c1],
                        start=(dt_ == 0), stop=(dt_ == NDT - 1),
                    )
                    yield
                nc.vector.tensor_scalar_add(dst[:, c0:c1], ps[:], bias_ap)

        def v1_make(st, kts):
            kvT = st["kvT"]
            for kt in kts:
                pst = psap.tile([128, 64], BF16, tag="psa", name=f"pst_{nc.next_id()}")
                nc.tensor.transpose(
                    pst[:], kvT[64:128, kt * 128:(kt + 1) * 128],
                    ident[64:128, 64:128],
                )
                v = v1p.tile([128, 65], BF16, tag="v1", name=f"v1_{nc.next_id()}")
                nc.vector.tensor_copy(v[:, 0:64], pst[:])
                nc.vector.memset(v[:, 64:65], 1.0)
                st["v1"][kt] = v

        def alloc_ab(b, st):
            st["qT"] = [qtp.tile([128, S], BF16, tag="qt", name=f"qT{b}_{i}")
                        for i in range(2)]
            st["kvT"] = kvp.tile([128, S], BF16, tag="kv", name=f"kvT{b}")
            st["v1"] = [None] * NKT

        def phase_a_rest(b, st):
            """KV slices 1-3 (+v1, kth), q1 sl0-1, q0 sl2-3, q1 sl2-3.
            Ordered by when phase C (sq-outer, h-inner) needs them."""
            qT, kvT = st["qT"], st["kvT"]
            for sl in (1, 2, 3):
                yield from proj(kvT, lambda dt_: dt_ * 128, wkv_sb,
                                bkv_sb[:], [sl])
                v1_make(st, range(4 * sl, 4 * sl + 4))
            kth = kthp.tile([128, S], BF16, tag="kth", name=f"kth_{nc.next_id()}")
            st["kth"] = kth
            nc.sync.dma_start(out=kth[64:128, :], in_=kvT[0:64, :])
            yield from proj(qT[1], lambda dt_: dt_ * 256 + 128, wq_sb,
                            bq_sb[:, 1:2], [0, 1])
            yield from proj(qT[0], lambda dt_: dt_ * 256, wq_sb,
                            bq_sb[:, 0:1], [2, 3])
            yield from proj(qT[1], lambda dt_: dt_ * 256 + 128, wq_sb,
                            bq_sb[:, 1:2], [2, 3])

        def phase_a1(st):
            hst_dma(1, 0)
            hst_dma(1, 1)
            yield
            alloc_ab(1, st)
            qT, kvT = st["qT"], st["kvT"]
            for sl in range(NSL):
                yield from proj(kvT, lambda dt_: dt_ * 128, wkv_sb,
                                bkv_sb[:], [sl])
                v1_make(st, range(4 * sl, 4 * sl + 4))
            kth = kthp.tile([128, S], BF16, tag="kth", name=f"kth_{nc.next_id()}")
            st["kth"] = kth
            nc.sync.dma_start(out=kth[64:128, :], in_=kvT[0:64, :])
            yield from proj(qT[0], lambda dt_: dt_ * 256, wq_sb,
                            bq_sb[:, 0:1], range(NSL))
            yield from proj(qT[1], lambda dt_: dt_ * 256 + 128, wq_sb,
                            bq_sb[:, 1:2], range(NSL))

        def run_gen(g, n):
            for _ in range(n):
                if g is None:
                    return None
                try:
                    next(g)
                except StopIteration:
                    return None
            return g

        # ---- Phase C (per batch): flat pipelined schedule over
        # g = (iter, kt) with iter = (sq, h).
        def phase_c(b, st, interleave, rate):
            # kth/v1 are created lazily by the interleaved phase-A generator,
            # always before the emission step that references them.
            qT, kvT, v1 = st["qT"], st["kvT"], st["v1"]
            iters = [(sq, h) for sq in range(NSQ) for h in range(QH)]
            nstep = len(iters) * NKT
            outt = [outp.tile([128, MCOLS], F32, tag="out",
                              name=f"outt{b}_{i}") for i in range(16)]
            prev = None   # (ex_tile, kt, ctx_tile)
            cur_ctx = None

            def scores(g):
                nonlocal cur_ctx
                it, kt = divmod(g, NKT)
                sq, h = iters[it]
                qrow = (h % 2) * 64
                kmat = kvT if qrow == 0 else st["kth"]
                qt = qT[h // 2]
                pss = pssp.tile([128, 1024], F32, tag="pss", name=f"pss_{nc.next_id()}")
                q0 = sq * 1024
                for qc in range(2):
                    nc.tensor.matmul(
                        pss[:, qc * 512:(qc + 1) * 512],
                        kmat[qrow:qrow + 64, kt * 128:(kt + 1) * 128],
                        qt[qrow:qrow + 64, q0 + qc * 512:q0 + (qc + 1) * 512],
                        start=True, stop=True,
                    )
                if kt == 0:
                    cur_ctx = ctxp.tile([128, 1024], F32, tag="ctx", name=f"ctx_{nc.next_id()}")
                return pss

            def expi(pss):
                ex = expp.tile([128, 1024], BF16, tag="exp", name=f"ex_{nc.next_id()}")
                nc.scalar.activation(ex[:], pss[:], Exp, bias=zb[:])
                return ex

            def pv(state):
                # start marks the whole 2KB psum bank pending-zero, so only
                # the first block per bank starts the group; the other
                # blocks' first writes land on pending bytes and overwrite.
                ex, kt, ctx_t = state
                for blk in range(8):
                    nc.tensor.matmul(
                        ctx_t[:, blk * 128:blk * 128 + 65],
                        ex[:, blk * 128:(blk + 1) * 128],
                        v1[kt][:],
                        start=(kt == 0 and blk % 4 == 0),
                        stop=(kt == NKT - 1 and blk % 4 == 3),
                    )

            def norm(it, ctx_t):
                sq, h = iters[it]
                for blk in range(8):
                    rec = recp.tile([128, 1], F32, tag="rec", name=f"rec_{nc.next_id()}")
                    nc.vector.reciprocal(
                        rec[:], ctx_t[:, blk * 128 + 64:blk * 128 + 65])
                    st_i = sq * 8 + blk
                    nc.vector.tensor_scalar_mul(
                        outt[st_i][:, h * 64:(h + 1) * 64],
                        ctx_t[:, blk * 128:blk * 128 + 64], rec[:],
                    )
                    if h == QH - 1:
                        nc.sync.dma_start(
                            out=out_d[b, st_i * 128:(st_i + 1) * 128, :],
                            in_=outt[st_i][:],
                        )

            for g in range(nstep):
                pss = scores(g)
                if prev is not None:
                    pv(prev)
                    if prev[1] == NKT - 1:
                        norm(g // NKT - 1, prev[2])
                prev = (expi(pss), g % NKT, cur_ctx)
                interleave = run_gen(interleave, rate(g))
            pv(prev)
            norm(len(iters) - 1, prev[2])
            return interleave

        def chain(*gens):
            for g in gens:
                yield from g

        st0, st1 = {}, {}
        alloc_ab(0, st0)
        # Eager startup: kv slice 0, its v1 tiles, q0 slices 0-1.
        for _ in proj(st0["kvT"], lambda dt_: dt_ * 128, wkv_sb,
                      bkv_sb[:], [0]):
            pass
        v1_make(st0, range(4))
        for _ in proj(st0["qT"][0], lambda dt_: dt_ * 256, wq_sb,
                      bq_sb[:, 0:1], [0, 1]):
            pass

        rest = chain(phase_a_rest(0, st0), phase_a1(st1))
        # Higher interleave rate early: kv slice sl (16 matmuls + bias-add)
        # must be fully EMITTED before scores(kt=4*sl) — emission order is
        # dependency order. kth must land before step 16 (sq0,h1).
        rest = phase_c(0, st0, rest, lambda g: 6 if g < 8 else (4 if g < 24 else 2))
        rest = phase_c(1, st1, rest, lambda g: 2)
        while rest is not None:          # safety net; normally exhausted
            rest = run_gen(rest, 16)

    nc.compile()
    return nc


def make_in_maps(hidden_states, Wq, bq, Wk, bk, Wv, bv):
    hs = np.asarray(hidden_states, dtype=np.float32)
    hst = np.ascontiguousarray(hs.transpose(0, 2, 1)).astype(BF16_NP)
    Wq = np.asarray(Wq, dtype=np.float32)
    bq = np.asarray(bq, dtype=np.float32)
    Wk = np.asarray(Wk, dtype=np.float32)
    bk = np.asarray(bk, dtype=np.float32)
    Wv = np.asarray(Wv, dtype=np.float32)
    bv = np.asarray(bv, dtype=np.float32)
    sc = 1.0 / np.sqrt(np.float32(HD))
    ident = np.eye(128, dtype=np.float32).astype(BF16_NP)
    in_maps = []
    for c in range(NCORES):
        qs = slice(c * MCOLS, (c + 1) * MCOLS)
        ks = slice(c * HD, (c + 1) * HD)
        bq_c = (bq[qs] * sc).reshape(2, 128).T
        in_maps.append({
            "hst": hst,
            "wq": np.ascontiguousarray(Wq[:, qs] * sc).astype(BF16_NP),
            "wkv": np.ascontiguousarray(
                np.concatenate([Wk[:, ks], Wv[:, ks]], axis=1)).astype(BF16_NP),
            "bq": np.ascontiguousarray(bq_c),
            "bkv": np.concatenate([bk[ks], bv[ks]]).reshape(128, 1),
            "ident": ident,
        })
    return in_maps


_NC_CACHE = {}


def get_nc():
    if "nc" not in _NC_CACHE:
        _NC_CACHE["nc"] = build_nc()
    return _NC_CACHE["nc"]


def kernel(hidden_states, Wq, bq, Wk, bk, Wv, bv):
    nc = get_nc()
    in_maps = make_in_maps(hidden_states, Wq, bq, Wk, bk, Wv, bv)
    res = run_bass_kernel_spmd(nc, in_maps, list(range(NCORES)))
    outs = [np.asarray(r["out"], dtype=np.float32) for r in res.results]
    return np.concatenate(outs, axis=-1)


# revision 14
# speedup vs baseline: 1.8576x; 1.0485x over previous
"""Trainium2 Bass kernel for GroupedQueryAttention (v3).

Sharding: 8 cores; core c owns KV head g=c and Q heads 4c..4c+3, both batch
elements. Each core computes its [2, 2048, 256] output slice; host concats.

Design:
  - hs is pre-transposed on the host to [B, D, S] and cast to bf16, so the
    kernel DMAs hs^T directly: no PE transposes / PSUM->SBUF copies for the
    activations, and half the HBM traffic. One big SBUF tile per batch
    (reused) loaded by 2 wide DMAs (HWDGE issue cost is per-DMA).
  - All matmul operands are bf16 (fp32 PSUM accumulation). bf16 runs at
    1 col/cycle on the PE regardless of output width.
  - PV in natural orientation: ctx[s_q, 0:65] accumulates lhsT=exp(S^T)
    block [128k, 128sq], rhs=[V|1] [128k, 65] over k tiles. The ones column
    yields the softmax denominator; DVE reciprocal-scales into the output
    tile. No ctx transposes. Only the first/last psum block per 2KB bank
    carries start/stop (start marks the whole bank pending-zero).
  - Phase C is a flat software-pipelined schedule over (sq, h, kt):
    scores(g) -> PV(g-1) -> exp(g), so ACT (the exp roofline, ~266us)
    streams back-to-back. Projections for later q blocks and for batch 1
    are interleaved into the attention stream via generators sized to the
    PE slack per step.
"""

import sys
from contextlib import ExitStack

import numpy as np

sys.path.insert(0, "/opt/trn_rl_repo")

import concourse.bass as bass  # noqa: E402
import concourse.bacc as bacc  # noqa: E402
import concourse.tile as tile  # noqa: E402
from concourse import mybir  # noqa: E402
from concourse.bass_utils import run_bass_kernel_spmd  # noqa: E402

B = 2
S = 2048
D = 2048
HD = 64
NCORES = 8
QH = 4           # q heads per core
MCOLS = QH * HD  # 256 output cols per core

BF16 = mybir.dt.bfloat16
F32 = mybir.dt.float32
Exp = mybir.ActivationFunctionType.Exp

NDT = 16         # d tiles of 128
NKT = 16         # s_k tiles of 128
NSQ = 2          # s_q chunks of 1024
NSL = 4          # s slices of 512 for projections

BF16_NP = mybir.dt.np(BF16)


def build_nc():
    nc = bacc.Bacc("TRN2", target_bir_lowering=False, debug=False)

    hst_d = nc.dram_tensor("hst", [B, D, S], BF16, kind="ExternalInput")
    wq_d = nc.dram_tensor("wq", [128, NDT * MCOLS], BF16, kind="ExternalInput")
    wkv_d = nc.dram_tensor("wkv", [128, NDT * 128], BF16, kind="ExternalInput")
    bq_d = nc.dram_tensor("bq", [128, 2], F32, kind="ExternalInput")
    bkv_d = nc.dram_tensor("bkv", [128, 1], F32, kind="ExternalInput")
    id_d = nc.dram_tensor("ident", [128, 128], BF16, kind="ExternalInput")
    out_d = nc.dram_tensor("out", [B, S, MCOLS], F32, kind="ExternalOutput")

    with tile.TileContext(nc) as tc, ExitStack() as ctx:
        const = ctx.enter_context(tc.tile_pool(name="const", bufs=1))
        hstp = ctx.enter_context(tc.tile_pool(name="hstp", bufs=1))
        qtp = ctx.enter_context(tc.tile_pool(name="qtp", bufs=4))
        kvp = ctx.enter_context(tc.tile_pool(name="kvp", bufs=2))
        kthp = ctx.enter_context(tc.tile_pool(name="kthp", bufs=2))
        v1p = ctx.enter_context(tc.tile_pool(name="v1p", bufs=2 * NKT))
        expp = ctx.enter_context(tc.tile_pool(name="expp", bufs=6))
        recp = ctx.enter_context(tc.tile_pool(name="recp", bufs=8))
        outp = ctx.enter_context(tc.tile_pool(name="outp", bufs=32))
        pssp = ctx.enter_context(tc.tile_pool(name="pssp", bufs=2, space="PSUM"))
        ctxp = ctx.enter_context(tc.tile_pool(name="ctxp", bufs=1, space="PSUM"))
        psap = ctx.enter_context(tc.tile_pool(name="psap", bufs=2, space="PSUM"))

        ident = const.tile([128, 128], BF16, tag="ident")
        bq_sb = const.tile([128, 2], F32, tag="bq")
        bkv_sb = const.tile([128, 1], F32, tag="bkv")
        zb = const.tile([128, 1], F32, tag="zb")
        nc.vector.memset(zb[:], 0.0)
        wq_sb = const.tile([128, NDT * MCOLS], BF16, tag="wq")
        wkv_sb = const.tile([128, NDT * 128], BF16, tag="wkv")

        hsT = const.tile([128, NDT * S], BF16, tag="hst")

        def hst_dma(b, c0, c1):
            nc.sync.dma_start(
                out=hsT[:].rearrange("p (t s) -> p t s", t=NDT)[:, :, c0:c1],
                in_=hst_d[b].rearrange("(t p) s -> p t s", p=128)[:, :, c0:c1],
            )

        # DMA order = arrival order (the cost model serializes transfers):
        # consts, wkv, wq, then batch 0's hs^T by s-slice so projections can
        # chase the arrivals.
        nc.sync.dma_start(out=ident[:], in_=id_d[:])
        nc.sync.dma_start(out=bq_sb[:], in_=bq_d[:])
        nc.sync.dma_start(out=bkv_sb[:], in_=bkv_d[:])
        nc.sync.dma_start(out=wkv_sb[:], in_=wkv_d[:])
        nc.sync.dma_start(out=wq_sb[:], in_=wq_d[:])
        for sl in range(NSL):
            hst_dma(0, sl * 512, (sl + 1) * 512)

        # Warm-up fillers: keep the PE continuously busy until the first
        # hs^T slice lands, so the p-state ramp (0.65->2.4GHz over 3us of
        # sustained use) completes before the real projections run.
        warm = const.tile([128, 640], BF16, tag="warm")
        nc.vector.memset(warm[:], 0.0)
        for i in range(40):
            wps = psap.tile([128, 512], F32, tag="psa", name=f"warm_{i}")
            nc.tensor.matmul(wps[:], warm[:, 0:128], warm[:, 128:640],
                             start=True, stop=True)

        # ---- Phase A pieces (generators; one yield per PE matmul).
        def proj(dst, lhs_col0, lhs_w, bias_ap, slices):
            for sl in slices:
                c0, c1 = sl * 512, (sl + 1) * 512
                ps = psap.tile([128, 512], F32, tag="psa", name=f"psa_{nc.next_id()}")
                for dt_ in range(NDT):
                    nc.tensor.matmul(
                        ps[:],
                        lhs_w[:, lhs_col0(dt_):lhs_col0(dt_) + 128],
                        hsT[:, dt_ * S + c0:dt_ * S + c1],
                        start=(dt_ == 0), stop=(dt_ == NDT - 1),
                    )
                    yield
                nc.vector.tensor_scalar_add(dst[:, c0:c1], ps[:], bias_ap)

        def v1_make(st, kts):
            kvT = st["kvT"]
            for kt in kts:
                pst = psap.tile([128, 64], BF16, tag="psa", name=f"pst_{nc.next_id()}")
                nc.tensor.transpose(
                    pst[:], kvT[64:128, kt * 128:(kt + 1) * 128],
                    ident[64:128, 64:128],
                )
                v = v1p.tile([128, 65], BF16, tag="v1", name=f"v1_{nc.next_id()}")
                nc.vector.tensor_copy(v[:, 0:64], pst[:])
                nc.vector.memset(v[:, 64:65], 1.0)
                st["v1"][kt] = v

        def alloc_ab(b, st):
            st["qT"] = [qtp.tile([128, S], BF16, tag="qt", name=f"qT{b}_{i}")
                        for i in range(2)]
            st["kvT"] = kvp.tile([128, S], BF16, tag="kv", name=f"kvT{b}")
            st["v1"] = [None] * NKT

        def phase_a_rest(b, st):
            """KV slices 1-3 (+v1, kth), q1 sl0-1, q0 sl2-3, q1 sl2-3.
            Ordered by when phase C (sq-outer, h-inner) needs them."""
            qT, kvT = st["qT"], st["kvT"]
            for sl in (1, 2, 3):
                yield from proj(kvT, lambda dt_: dt_ * 128, wkv_sb,
                                bkv_sb[:], [sl])
                v1_make(st, range(4 * sl, 4 * sl + 4))
            kth = kthp.tile([128, S], BF16, tag="kth", name=f"kth_{nc.next_id()}")
            st["kth"] = kth
            nc.sync.dma_start(out=kth[64:128, :], in_=kvT[0:64, :])
            yield from proj(qT[1], lambda dt_: dt_ * 256 + 128, wq_sb,
                            bq_sb[:, 1:2], [0, 1])
            yield from proj(qT[0], lambda dt_: dt_ * 256, wq_sb,
                            bq_sb[:, 0:1], [2, 3])
            yield from proj(qT[1], lambda dt_: dt_ * 256 + 128, wq_sb,
                            bq_sb[:, 1:2], [2, 3])

        def phase_a1(st):
            hst_dma(1, 0, 1024)
            hst_dma(1, 1024, 2048)
            yield
            alloc_ab(1, st)
            qT, kvT = st["qT"], st["kvT"]
            for sl in range(NSL):
                yield from proj(kvT, lambda dt_: dt_ * 128, wkv_sb,
                                bkv_sb[:], [sl])
                v1_make(st, range(4 * sl, 4 * sl + 4))
            kth = kthp.tile([128, S], BF16, tag="kth", name=f"kth_{nc.next_id()}")
            st["kth"] = kth
            nc.sync.dma_start(out=kth[64:128, :], in_=kvT[0:64, :])
            yield from proj(qT[0], lambda dt_: dt_ * 256, wq_sb,
                            bq_sb[:, 0:1], range(NSL))
            yield from proj(qT[1], lambda dt_: dt_ * 256 + 128, wq_sb,
                            bq_sb[:, 1:2], range(NSL))

        def run_gen(g, n):
            for _ in range(n):
                if g is None:
                    return None
                try:
                    next(g)
                except StopIteration:
                    return None
            return g

        # ---- Phase C (per batch): flat pipelined schedule over
        # g = (iter, kt) with iter = (sq, h).
        def phase_c(b, st, interleave, rate):
            # kth/v1 are created lazily by the interleaved phase-A generator,
            # always before the emission step that references them.
            qT, kvT, v1 = st["qT"], st["kvT"], st["v1"]
            iters = [(sq, h) for sq in range(NSQ) for h in range(QH)]
            nstep = len(iters) * NKT
            outt = [outp.tile([128, MCOLS], F32, tag="out",
                              name=f"outt{b}_{i}") for i in range(16)]
            prev = None   # (ex_tile, kt, ctx_tile)
            cur_ctx = None

            def scores(g):
                nonlocal cur_ctx
                it, kt = divmod(g, NKT)
                sq, h = iters[it]
                qrow = (h % 2) * 64
                kmat = kvT if qrow == 0 else st["kth"]
                qt = qT[h // 2]
                pss = pssp.tile([128, 1024], F32, tag="pss", name=f"pss_{nc.next_id()}")
                q0 = sq * 1024
                for qc in range(2):
                    nc.tensor.matmul(
                        pss[:, qc * 512:(qc + 1) * 512],
                        kmat[qrow:qrow + 64, kt * 128:(kt + 1) * 128],
                        qt[qrow:qrow + 64, q0 + qc * 512:q0 + (qc + 1) * 512],
                        start=True, stop=True,
                    )
                if kt == 0:
                    cur_ctx = ctxp.tile([128, 1024], F32, tag="ctx", name=f"ctx_{nc.next_id()}")
                return pss

            def expi(pss):
                ex = expp.tile([128, 1024], BF16, tag="exp", name=f"ex_{nc.next_id()}")
                nc.scalar.activation(ex[:], pss[:], Exp, bias=zb[:])
                return ex

            def pv(state):
                # start marks the whole 2KB psum bank pending-zero, so only
                # the first block per bank starts the group; the other
                # blocks' first writes land on pending bytes and overwrite.
                ex, kt, ctx_t = state
                for blk in range(8):
                    nc.tensor.matmul(
                        ctx_t[:, blk * 128:blk * 128 + 65],
                        ex[:, blk * 128:(blk + 1) * 128],
                        v1[kt][:],
                        start=(kt == 0 and blk % 4 == 0),
                        stop=(kt == NKT - 1 and blk % 4 == 3),
                    )

            def norm(it, ctx_t):
                sq, h = iters[it]
                for blk in range(8):
                    rec = recp.tile([128, 1], F32, tag="rec", name=f"rec_{nc.next_id()}")
                    nc.vector.reciprocal(
                        rec[:], ctx_t[:, blk * 128 + 64:blk * 128 + 65])
                    st_i = sq * 8 + blk
                    nc.vector.tensor_scalar_mul(
                        outt[st_i][:, h * 64:(h + 1) * 64],
                        ctx_t[:, blk * 128:blk * 128 + 64], rec[:],
                    )
                    if h == QH - 1:
                        nc.sync.dma_start(
                            out=out_d[b, st_i * 128:(st_i + 1) * 128, :],
                            in_=outt[st_i][:],
                        )

            for g in range(nstep):
                pss = scores(g)
                ex = expi(pss)
                if prev is not None:
                    pv(prev)
                    if prev[1] == NKT - 1:
                        norm(g // NKT - 1, prev[2])
                prev = (ex, g % NKT, cur_ctx)
                interleave = run_gen(interleave, rate(g))
            pv(prev)
            norm(len(iters) - 1, prev[2])
            return interleave

        def chain(*gens):
            for g in gens:
                yield from g

        st0, st1 = {}, {}
        alloc_ab(0, st0)
        # Eager startup: kv slice 0, its v1 tiles, q0 slices 0-1.
        for _ in proj(st0["kvT"], lambda dt_: dt_ * 128, wkv_sb,
                      bkv_sb[:], [0]):
            pass
        v1_make(st0, range(4))
        for _ in proj(st0["qT"][0], lambda dt_: dt_ * 256, wq_sb,
                      bq_sb[:, 0:1], [0, 1]):
            pass

        rest = chain(phase_a_rest(0, st0), phase_a1(st1))
        # Interleave pacing: kv slice sl must be fully EMITTED before
        # scores(kt=4*sl) (emission order is dependency order), but not so
        # early that the PE stalls on its hs^T slice still in flight.
        def rate0(g):
            if g < 2:
                return 8
            if g < 5:
                return 6
            if g < 7:
                return 3
            if g < 11:
                return 5
            return 2
        rest = phase_c(0, st0, rest, rate0)
        rest = phase_c(1, st1, rest, lambda g: 2)
        while rest is not None:          # safety net; normally exhausted
            rest = run_gen(rest, 16)

    nc.compile()
    return nc


def make_in_maps(hidden_states, Wq, bq, Wk, bk, Wv, bv):
    hs = np.asarray(hidden_states, dtype=np.float32)
    hst = np.ascontiguousarray(hs.transpose(0, 2, 1)).astype(BF16_NP)
    Wq = np.asarray(Wq, dtype=np.float32)
    bq = np.asarray(bq, dtype=np.float32)
    Wk = np.asarray(Wk, dtype=np.float32)
    bk = np.asarray(bk, dtype=np.float32)
    Wv = np.asarray(Wv, dtype=np.float32)
    bv = np.asarray(bv, dtype=np.float32)
    sc = 1.0 / np.sqrt(np.float32(HD))
    ident = np.eye(128, dtype=np.float32).astype(BF16_NP)

    def tile_weights(w):
        # [D, C] -> [128, NDT*C] with layout [p, t*C + c] = w[t*128+p, c]
        cdim = w.shape[1]
        return np.ascontiguousarray(
            w.reshape(NDT, 128, cdim).transpose(1, 0, 2).reshape(128, -1)
        ).astype(BF16_NP)

    in_maps = []
    for c in range(NCORES):
        qs = slice(c * MCOLS, (c + 1) * MCOLS)
        ks = slice(c * HD, (c + 1) * HD)
        bq_c = (bq[qs] * sc).reshape(2, 128).T
        in_maps.append({
            "hst": hst,
            "wq": tile_weights(Wq[:, qs] * sc),
            "wkv": tile_weights(
                np.concatenate([Wk[:, ks], Wv[:, ks]], axis=1)),
            "bq": np.ascontiguousarray(bq_c),
            "bkv": np.concatenate([bk[ks], bv[ks]]).reshape(128, 1),
            "ident": ident,
        })
    return in_maps


_NC_CACHE = {}


def get_nc():
    if "nc" not in _NC_CACHE:
        _NC_CACHE["nc"] = build_nc()
    return _NC_CACHE["nc"]


def kernel(hidden_states, Wq, bq, Wk, bk, Wv, bv):
    nc = get_nc()
    in_maps = make_in_maps(hidden_states, Wq, bq, Wk, bk, Wv, bv)
    res = run_bass_kernel_spmd(nc, in_maps, list(range(NCORES)))
    outs = [np.asarray(r["out"], dtype=np.float32) for r in res.results]
    return np.concatenate(outs, axis=-1)


# revision 17
# speedup vs baseline: 1.9087x; 1.0275x over previous
"""Trainium2 Bass kernel for GroupedQueryAttention (v3).

Sharding: 8 cores; core c owns KV head g=c and Q heads 4c..4c+3, both batch
elements. Each core computes its [2, 2048, 256] output slice; host concats.

Design:
  - hs is pre-transposed on the host to [B, D, S] and cast to bf16, so the
    kernel DMAs hs^T directly: no PE transposes / PSUM->SBUF copies for the
    activations, and half the HBM traffic. One big SBUF tile per batch
    (reused) loaded by 2 wide DMAs (HWDGE issue cost is per-DMA).
  - All matmul operands are bf16 (fp32 PSUM accumulation). bf16 runs at
    1 col/cycle on the PE regardless of output width.
  - PV in natural orientation: ctx[s_q, 0:65] accumulates lhsT=exp(S^T)
    block [128k, 128sq], rhs=[V|1] [128k, 65] over k tiles. The ones column
    yields the softmax denominator; DVE reciprocal-scales into the output
    tile. No ctx transposes. Only the first/last psum block per 2KB bank
    carries start/stop (start marks the whole bank pending-zero).
  - Phase C is a flat software-pipelined schedule over (sq, h, kt):
    scores(g) -> PV(g-1) -> exp(g), so ACT (the exp roofline, ~266us)
    streams back-to-back. Projections for later q blocks and for batch 1
    are interleaved into the attention stream via generators sized to the
    PE slack per step.
"""

import sys
from contextlib import ExitStack

import numpy as np

sys.path.insert(0, "/opt/trn_rl_repo")

import concourse.bass as bass  # noqa: E402
import concourse.bacc as bacc  # noqa: E402
import concourse.tile as tile  # noqa: E402
from concourse import mybir  # noqa: E402
from concourse.bass_utils import run_bass_kernel_spmd  # noqa: E402

B = 2
S = 2048
D = 2048
HD = 64
NCORES = 8
QH = 4           # q heads per core
MCOLS = QH * HD  # 256 output cols per core

BF16 = mybir.dt.bfloat16
F32 = mybir.dt.float32
Exp = mybir.ActivationFunctionType.Exp

NDT = 16         # d tiles of 128
NKT = 16         # s_k tiles of 128
NSQ = 2          # s_q chunks of 1024
NSL = 4          # s slices of 512 for projections

BF16_NP = mybir.dt.np(BF16)


def build_nc():
    nc = bacc.Bacc("TRN2", target_bir_lowering=False, debug=False)

    hst_d = nc.dram_tensor("hst", [B, D, S], BF16, kind="ExternalInput")
    wq_d = nc.dram_tensor("wq", [128, NDT * MCOLS], BF16, kind="ExternalInput")
    wkv_d = nc.dram_tensor("wkv", [128, NDT * 128], BF16, kind="ExternalInput")
    bq_d = nc.dram_tensor("bq", [128, 2], F32, kind="ExternalInput")
    bkv_d = nc.dram_tensor("bkv", [128, 1], F32, kind="ExternalInput")
    id_d = nc.dram_tensor("ident", [128, 128], BF16, kind="ExternalInput")
    out_d = nc.dram_tensor("out", [B, S, MCOLS], F32, kind="ExternalOutput")

    with tile.TileContext(nc) as tc, ExitStack() as ctx:
        const = ctx.enter_context(tc.tile_pool(name="const", bufs=1))
        hstp = ctx.enter_context(tc.tile_pool(name="hstp", bufs=1))
        qtp = ctx.enter_context(tc.tile_pool(name="qtp", bufs=4))
        kvp = ctx.enter_context(tc.tile_pool(name="kvp", bufs=2))
        kthp = ctx.enter_context(tc.tile_pool(name="kthp", bufs=2))
        v1p = ctx.enter_context(tc.tile_pool(name="v1p", bufs=2 * NKT))
        expp = ctx.enter_context(tc.tile_pool(name="expp", bufs=6))
        recp = ctx.enter_context(tc.tile_pool(name="recp", bufs=8))
        outp = ctx.enter_context(tc.tile_pool(name="outp", bufs=32))
        pssp = ctx.enter_context(tc.tile_pool(name="pssp", bufs=2, space="PSUM"))
        ctxp = ctx.enter_context(tc.tile_pool(name="ctxp", bufs=1, space="PSUM"))
        psap = ctx.enter_context(tc.tile_pool(name="psap", bufs=2, space="PSUM"))

        ident = const.tile([128, 128], BF16, tag="ident")
        bq_sb = const.tile([128, 2], F32, tag="bq")
        bkv_sb = const.tile([128, 1], F32, tag="bkv")
        zb = const.tile([128, 1], F32, tag="zb")
        nc.vector.memset(zb[:], 0.0)
        wq_sb = const.tile([128, NDT * MCOLS], BF16, tag="wq")
        wkv_sb = const.tile([128, NDT * 128], BF16, tag="wkv")

        hsT = const.tile([128, NDT * S], BF16, tag="hst")

        def hst_dma(b, c0, c1):
            nc.sync.dma_start(
                out=hsT[:].rearrange("p (t s) -> p t s", t=NDT)[:, :, c0:c1],
                in_=hst_d[b].rearrange("(t p) s -> p t s", p=128)[:, :, c0:c1],
            )

        # DMA order = arrival order (the cost model serializes transfers):
        # consts, wkv, wq, then batch 0's hs^T by s-slice so projections can
        # chase the arrivals.
        nc.sync.dma_start(out=ident[:], in_=id_d[:])
        nc.sync.dma_start(out=bq_sb[:], in_=bq_d[:])
        nc.sync.dma_start(out=bkv_sb[:], in_=bkv_d[:])
        nc.sync.dma_start(out=wkv_sb[:], in_=wkv_d[:])
        hst_dma(0, 0, 512)          # kv slice 0 only needs wkv + this
        nc.sync.dma_start(out=wq_sb[:], in_=wq_d[:])
        for sl in range(1, NSL):
            hst_dma(0, sl * 512, (sl + 1) * 512)

        # Warm-up fillers: keep the PE continuously busy until the first
        # hs^T slice lands, so the p-state ramp (0.65->2.4GHz over 3us of
        # sustained use) completes before the real projections run.
        warm = const.tile([128, 640], BF16, tag="warm")
        nc.vector.memset(warm[:], 0.0)
        for i in range(34):
            wps = psap.tile([128, 512], F32, tag="psa", name=f"warm_{i}")
            nc.tensor.matmul(wps[:], warm[:, 0:128], warm[:, 128:640],
                             start=True, stop=True)

        # ---- Phase A pieces (generators; one yield per PE matmul).
        def proj(dst, lhs_col0, lhs_w, bias_ap, slices):
            for sl in slices:
                c0, c1 = sl * 512, (sl + 1) * 512
                ps = psap.tile([128, 512], F32, tag="psa", name=f"psa_{nc.next_id()}")
                for dt_ in range(NDT):
                    nc.tensor.matmul(
                        ps[:],
                        lhs_w[:, lhs_col0(dt_):lhs_col0(dt_) + 128],
                        hsT[:, dt_ * S + c0:dt_ * S + c1],
                        start=(dt_ == 0), stop=(dt_ == NDT - 1),
                    )
                    yield
                nc.vector.tensor_scalar_add(dst[:, c0:c1], ps[:], bias_ap)

        def v1_make(st, kts):
            kvT = st["kvT"]
            for kt in kts:
                pst = psap.tile([128, 64], BF16, tag="psa", name=f"pst_{nc.next_id()}")
                nc.tensor.transpose(
                    pst[:], kvT[64:128, kt * 128:(kt + 1) * 128],
                    ident[64:128, 64:128],
                )
                v = v1p.tile([128, 65], BF16, tag="v1", name=f"v1_{nc.next_id()}")
                nc.vector.tensor_copy(v[:, 0:64], pst[:])
                nc.vector.memset(v[:, 64:65], 1.0)
                st["v1"][kt] = v

        def alloc_ab(b, st):
            st["qT"] = [qtp.tile([128, S], BF16, tag="qt", name=f"qT{b}_{i}")
                        for i in range(2)]
            st["kvT"] = kvp.tile([128, S], BF16, tag="kv", name=f"kvT{b}")
            st["v1"] = [None] * NKT

        def phase_a_rest(b, st):
            """KV slices 1-3 (+v1, kth), q1 sl0-1, q0 sl2-3, q1 sl2-3.
            Ordered by when phase C (sq-outer, h-inner) needs them."""
            qT, kvT = st["qT"], st["kvT"]
            for sl in (1, 2, 3):
                yield from proj(kvT, lambda dt_: dt_ * 128, wkv_sb,
                                bkv_sb[:], [sl])
                v1_make(st, range(4 * sl, 4 * sl + 4))
            kth = kthp.tile([128, S], BF16, tag="kth", name=f"kth_{nc.next_id()}")
            st["kth"] = kth
            nc.sync.dma_start(out=kth[64:128, :], in_=kvT[0:64, :])
            yield from proj(qT[1], lambda dt_: dt_ * 256 + 128, wq_sb,
                            bq_sb[:, 1:2], [0, 1])
            yield from proj(qT[0], lambda dt_: dt_ * 256, wq_sb,
                            bq_sb[:, 0:1], [2, 3])
            yield from proj(qT[1], lambda dt_: dt_ * 256 + 128, wq_sb,
                            bq_sb[:, 1:2], [2, 3])

        def phase_a1(st):
            for sl in range(NSL):
                hst_dma(1, sl * 512, (sl + 1) * 512)
            yield
            alloc_ab(1, st)
            qT, kvT = st["qT"], st["kvT"]
            for sl in range(NSL):
                yield from proj(kvT, lambda dt_: dt_ * 128, wkv_sb,
                                bkv_sb[:], [sl])
                v1_make(st, range(4 * sl, 4 * sl + 4))
            kth = kthp.tile([128, S], BF16, tag="kth", name=f"kth_{nc.next_id()}")
            st["kth"] = kth
            nc.sync.dma_start(out=kth[64:128, :], in_=kvT[0:64, :])
            yield from proj(qT[1], lambda dt_: dt_ * 256 + 128, wq_sb,
                            bq_sb[:, 1:2], [0, 1])
            yield from proj(qT[0], lambda dt_: dt_ * 256, wq_sb,
                            bq_sb[:, 0:1], [0, 1])
            yield from proj(qT[1], lambda dt_: dt_ * 256 + 128, wq_sb,
                            bq_sb[:, 1:2], [2, 3])
            yield from proj(qT[0], lambda dt_: dt_ * 256, wq_sb,
                            bq_sb[:, 0:1], [2, 3])

        def run_gen(g, n):
            for _ in range(n):
                if g is None:
                    return None
                try:
                    next(g)
                except StopIteration:
                    return None
            return g

        # ---- Phase C (per batch): flat pipelined schedule over
        # g = (iter, kt) with iter = (sq, h).
        def phase_c(b, st, interleave, rate, horder=(0, 1, 2, 3)):
            # kth/v1 are created lazily by the interleaved phase-A generator,
            # always before the emission step that references them.
            qT, kvT, v1 = st["qT"], st["kvT"], st["v1"]
            iters = [(sq, h) for sq in range(NSQ) for h in horder]
            heads_done = [0] * NSQ
            nstep = len(iters) * NKT
            outt = [outp.tile([128, MCOLS], F32, tag="out",
                              name=f"outt{b}_{i}") for i in range(16)]
            prev = None   # (ex_tile, kt, ctx_tile)
            cur_ctx = None

            def scores(g):
                nonlocal cur_ctx
                it, kt = divmod(g, NKT)
                sq, h = iters[it]
                qrow = (h % 2) * 64
                kmat = kvT if qrow == 0 else st["kth"]
                qt = qT[h // 2]
                pss = pssp.tile([128, 1024], F32, tag="pss", name=f"pss_{nc.next_id()}")
                q0 = sq * 1024
                for qc in range(2):
                    nc.tensor.matmul(
                        pss[:, qc * 512:(qc + 1) * 512],
                        kmat[qrow:qrow + 64, kt * 128:(kt + 1) * 128],
                        qt[qrow:qrow + 64, q0 + qc * 512:q0 + (qc + 1) * 512],
                        start=True, stop=True,
                    )
                if kt == 0:
                    cur_ctx = ctxp.tile([128, 1024], F32, tag="ctx", name=f"ctx_{nc.next_id()}")
                return pss

            def expi(pss):
                ex = expp.tile([128, 1024], BF16, tag="exp", name=f"ex_{nc.next_id()}")
                nc.scalar.activation(ex[:], pss[:], Exp, bias=zb[:])
                return ex

            def pv(state):
                # start marks the whole 2KB psum bank pending-zero, so only
                # the first block per bank starts the group; the other
                # blocks' first writes land on pending bytes and overwrite.
                ex, kt, ctx_t = state
                for blk in range(8):
                    nc.tensor.matmul(
                        ctx_t[:, blk * 128:blk * 128 + 65],
                        ex[:, blk * 128:(blk + 1) * 128],
                        v1[kt][:],
                        start=(kt == 0 and blk % 4 == 0),
                        stop=(kt == NKT - 1 and blk % 4 == 3),
                    )

            def norm(it, ctx_t):
                sq, h = iters[it]
                heads_done[sq] += 1
                flush = heads_done[sq] == QH
                for blk in range(8):
                    st_i = sq * 8 + blk
                    rec = recp.tile([128, 1], F32, tag="rec",
                                    name=f"rec_{nc.next_id()}")
                    nc.vector.reciprocal(
                        rec[:], ctx_t[:, blk * 128 + 64:blk * 128 + 65])
                    nc.vector.tensor_scalar_mul(
                        outt[st_i][:, h * 64:(h + 1) * 64],
                        ctx_t[:, blk * 128:blk * 128 + 64], rec[:],
                    )
                    if flush:
                        nc.sync.dma_start(
                            out=out_d[b, st_i * 128:(st_i + 1) * 128, :],
                            in_=outt[st_i][:],
                        )

            for g in range(nstep):
                pss = scores(g)
                ex = expi(pss)
                # interleave right after scores/exp: the PV and next scores
                # stay adjacent in PE order at iteration boundaries.
                interleave = run_gen(interleave, rate(g))
                if prev is not None:
                    pv(prev)
                    if prev[1] == NKT - 1:
                        norm(g // NKT - 1, prev[2])
                prev = (ex, g % NKT, cur_ctx)
            pv(prev)
            norm(len(iters) - 1, prev[2])
            return interleave

        def chain(*gens):
            for g in gens:
                yield from g

        st0, st1 = {}, {}
        alloc_ab(0, st0)
        # Eager startup: kv slice 0, q0 slices 0-1, then v1 tiles (v1 last:
        # its psum pool slots would otherwise gate q0's psum on DVE copies).
        for _ in proj(st0["kvT"], lambda dt_: dt_ * 128, wkv_sb,
                      bkv_sb[:], [0]):
            pass
        for _ in proj(st0["qT"][0], lambda dt_: dt_ * 256, wq_sb,
                      bq_sb[:, 0:1], [0, 1]):
            pass
        v1_make(st0, range(4))

        rest = chain(phase_a_rest(0, st0), phase_a1(st1))
        # Interleave pacing: kv slice sl must be fully EMITTED before
        # scores(kt=4*sl) (emission order is dependency order), but not so
        # early that the PE stalls on its hs^T slice still in flight.
        def rate0(g):
            if g < 2:
                return 8
            if g < 5:
                return 6
            if g < 7:
                return 3
            if g < 11:
                return 5
            return (2, 2, 1)[g % 3]
        rest = phase_c(0, st0, rest, rate0)
        rest = phase_c(1, st1, rest, lambda g: 2, horder=(2, 3, 0, 1))
        while rest is not None:          # safety net; normally exhausted
            rest = run_gen(rest, 16)

    nc.compile()
    return nc


def make_in_maps(hidden_states, Wq, bq, Wk, bk, Wv, bv):
    hs = np.asarray(hidden_states, dtype=np.float32)
    hst = np.ascontiguousarray(hs.transpose(0, 2, 1)).astype(BF16_NP)
    Wq = np.asarray(Wq, dtype=np.float32)
    bq = np.asarray(bq, dtype=np.float32)
    Wk = np.asarray(Wk, dtype=np.float32)
    bk = np.asarray(bk, dtype=np.float32)
    Wv = np.asarray(Wv, dtype=np.float32)
    bv = np.asarray(bv, dtype=np.float32)
    sc = 1.0 / np.sqrt(np.float32(HD))
    ident = np.eye(128, dtype=np.float32).astype(BF16_NP)

    def tile_weights(w):
        # [D, C] -> [128, NDT*C] with layout [p, t*C + c] = w[t*128+p, c]
        cdim = w.shape[1]
        return np.ascontiguousarray(
            w.reshape(NDT, 128, cdim).transpose(1, 0, 2).reshape(128, -1)
        ).astype(BF16_NP)

    in_maps = []
    for c in range(NCORES):
        qs = slice(c * MCOLS, (c + 1) * MCOLS)
        ks = slice(c * HD, (c + 1) * HD)
        bq_c = (bq[qs] * sc).reshape(2, 128).T
        in_maps.append({
            "hst": hst,
            "wq": tile_weights(Wq[:, qs] * sc),
            "wkv": tile_weights(
                np.concatenate([Wk[:, ks], Wv[:, ks]], axis=1)),
            "bq": np.ascontiguousarray(bq_c),
            "bkv": np.concatenate([bk[ks], bv[ks]]).reshape(128, 1),
            "ident": ident,
        })
    return in_maps


_NC_CACHE = {}


def get_nc():
    if "nc" not in _NC_CACHE:
        _NC_CACHE["nc"] = build_nc()
    return _NC_CACHE["nc"]


def kernel(hidden_states, Wq, bq, Wk, bk, Wv, bv):
    nc = get_nc()
    in_maps = make_in_maps(hidden_states, Wq, bq, Wk, bk, Wv, bv)
    res = run_bass_kernel_spmd(nc, in_maps, list(range(NCORES)))
    outs = [np.asarray(r["out"], dtype=np.float32) for r in res.results]
    return np.concatenate(outs, axis=-1)


# revision 18
# speedup vs baseline: 1.9436x; 1.0183x over previous
"""Trainium2 Bass kernel for GroupedQueryAttention (v3).

Sharding: 8 cores; core c owns KV head g=c and Q heads 4c..4c+3, both batch
elements. Each core computes its [2, 2048, 256] output slice; host concats.

Design:
  - hs is pre-transposed on the host to [B, D, S] and cast to bf16, so the
    kernel DMAs hs^T directly: no PE transposes / PSUM->SBUF copies for the
    activations, and half the HBM traffic. One big SBUF tile per batch
    (reused) loaded by 2 wide DMAs (HWDGE issue cost is per-DMA).
  - All matmul operands are bf16 (fp32 PSUM accumulation). bf16 runs at
    1 col/cycle on the PE regardless of output width.
  - PV in natural orientation: ctx[s_q, 0:65] accumulates lhsT=exp(S^T)
    block [128k, 128sq], rhs=[V|1] [128k, 65] over k tiles. The ones column
    yields the softmax denominator; DVE reciprocal-scales into the output
    tile. No ctx transposes. Only the first/last psum block per 2KB bank
    carries start/stop (start marks the whole bank pending-zero).
  - Phase C is a flat software-pipelined schedule over (sq, h, kt):
    scores(g) -> PV(g-1) -> exp(g), so ACT (the exp roofline, ~266us)
    streams back-to-back. Projections for later q blocks and for batch 1
    are interleaved into the attention stream via generators sized to the
    PE slack per step.
"""

import sys
from contextlib import ExitStack

import numpy as np

sys.path.insert(0, "/opt/trn_rl_repo")

import concourse.bass as bass  # noqa: E402
import concourse.bacc as bacc  # noqa: E402
import concourse.tile as tile  # noqa: E402
from concourse import mybir  # noqa: E402
from concourse.bass_utils import run_bass_kernel_spmd  # noqa: E402

B = 2
S = 2048
D = 2048
HD = 64
NCORES = 8
QH = 4           # q heads per core
MCOLS = QH * HD  # 256 output cols per core

BF16 = mybir.dt.bfloat16
F32 = mybir.dt.float32
Exp = mybir.ActivationFunctionType.Exp

NDT = 16         # d tiles of 128
NKT = 16         # s_k tiles of 128
NSQ = 2          # s_q chunks of 1024
NSL = 4          # s slices of 512 for projections

BF16_NP = mybir.dt.np(BF16)


def build_nc():
    nc = bacc.Bacc("TRN2", target_bir_lowering=False, debug=False)

    hst_d = nc.dram_tensor("hst", [B, D, S], BF16, kind="ExternalInput")
    wq_d = nc.dram_tensor("wq", [128, NDT * MCOLS], BF16, kind="ExternalInput")
    wkv_d = nc.dram_tensor("wkv", [128, NDT * 128], BF16, kind="ExternalInput")
    bq_d = nc.dram_tensor("bq", [128, 2], F32, kind="ExternalInput")
    bkv_d = nc.dram_tensor("bkv", [128, 1], F32, kind="ExternalInput")
    id_d = nc.dram_tensor("ident", [128, 128], BF16, kind="ExternalInput")
    out_d = nc.dram_tensor("out", [B, S, MCOLS], F32, kind="ExternalOutput")

    with tile.TileContext(nc) as tc, ExitStack() as ctx:
        const = ctx.enter_context(tc.tile_pool(name="const", bufs=1))
        hstp = ctx.enter_context(tc.tile_pool(name="hstp", bufs=1))
        qtp = ctx.enter_context(tc.tile_pool(name="qtp", bufs=4))
        kvp = ctx.enter_context(tc.tile_pool(name="kvp", bufs=2))
        kthp = ctx.enter_context(tc.tile_pool(name="kthp", bufs=2))
        v1p = ctx.enter_context(tc.tile_pool(name="v1p", bufs=2 * NKT))
        expp = ctx.enter_context(tc.tile_pool(name="expp", bufs=6))
        recp = ctx.enter_context(tc.tile_pool(name="recp", bufs=8))
        outp = ctx.enter_context(tc.tile_pool(name="outp", bufs=32))
        pssp = ctx.enter_context(tc.tile_pool(name="pssp", bufs=2, space="PSUM"))
        ctxp = ctx.enter_context(tc.tile_pool(name="ctxp", bufs=1, space="PSUM"))
        psap = ctx.enter_context(tc.tile_pool(name="psap", bufs=2, space="PSUM"))

        ident = const.tile([128, 128], BF16, tag="ident")
        bq_sb = const.tile([128, 2], F32, tag="bq")
        bkv_sb = const.tile([128, 1], F32, tag="bkv")
        zb = const.tile([128, 1], F32, tag="zb")
        nc.vector.memset(zb[:], 0.0)
        wq_sb = const.tile([128, NDT * MCOLS], BF16, tag="wq")
        wkv_sb = const.tile([128, NDT * 128], BF16, tag="wkv")

        hsT = const.tile([128, NDT * S], BF16, tag="hst")

        def hst_dma(b, c0, c1):
            nc.sync.dma_start(
                out=hsT[:].rearrange("p (t s) -> p t s", t=NDT)[:, :, c0:c1],
                in_=hst_d[b].rearrange("(t p) s -> p t s", p=128)[:, :, c0:c1],
            )

        # DMA order = arrival order (the cost model serializes transfers):
        # consts, wkv, wq, then batch 0's hs^T by s-slice so projections can
        # chase the arrivals.
        nc.sync.dma_start(out=ident[:], in_=id_d[:])
        nc.sync.dma_start(out=bq_sb[:], in_=bq_d[:])
        nc.sync.dma_start(out=bkv_sb[:], in_=bkv_d[:])
        nc.sync.dma_start(out=wkv_sb[:], in_=wkv_d[:])
        hst_dma(0, 0, 512)          # kv slice 0 only needs wkv + this
        nc.sync.dma_start(out=wq_sb[:], in_=wq_d[:])
        for sl in range(1, NSL):
            hst_dma(0, sl * 512, (sl + 1) * 512)

        # Warm-up fillers: keep the PE continuously busy until the first
        # hs^T slice lands, so the p-state ramp (0.65->2.4GHz over 3us of
        # sustained use) completes before the real projections run.
        warm = const.tile([128, 640], BF16, tag="warm")
        nc.vector.memset(warm[:], 0.0)
        for i in range(34):
            wps = psap.tile([128, 512], F32, tag="psa", name=f"warm_{i}")
            nc.tensor.matmul(wps[:], warm[:, 0:128], warm[:, 128:640],
                             start=True, stop=True)

        # ---- Phase A pieces (generators; one yield per PE matmul).
        def proj(dst, lhs_col0, lhs_w, bias_ap, slices):
            for sl in slices:
                c0, c1 = sl * 512, (sl + 1) * 512
                ps = psap.tile([128, 512], F32, tag="psa", name=f"psa_{nc.next_id()}")
                for dt_ in range(NDT):
                    nc.tensor.matmul(
                        ps[:],
                        lhs_w[:, lhs_col0(dt_):lhs_col0(dt_) + 128],
                        hsT[:, dt_ * S + c0:dt_ * S + c1],
                        start=(dt_ == 0), stop=(dt_ == NDT - 1),
                    )
                    yield
                nc.vector.tensor_scalar_add(dst[:, c0:c1], ps[:], bias_ap)

        def v1_make(st, kts):
            kvT = st["kvT"]
            for kt in kts:
                pst = psap.tile([128, 64], BF16, tag="psa", name=f"pst_{nc.next_id()}")
                nc.tensor.transpose(
                    pst[:], kvT[64:128, kt * 128:(kt + 1) * 128],
                    ident[64:128, 64:128],
                )
                v = v1p.tile([128, 65], BF16, tag="v1", name=f"v1_{nc.next_id()}")
                nc.vector.tensor_copy(v[:, 0:64], pst[:])
                nc.vector.memset(v[:, 64:65], 1.0)
                st["v1"][kt] = v

        def alloc_ab(b, st):
            st["qT"] = [qtp.tile([128, S], BF16, tag="qt", name=f"qT{b}_{i}")
                        for i in range(2)]
            st["kvT"] = kvp.tile([128, S], BF16, tag="kv", name=f"kvT{b}")
            st["v1"] = [None] * NKT

        def phase_a_rest(b, st):
            """KV slices 1-3 (+v1, kth), q1 sl0-1, q0 sl2-3, q1 sl2-3.
            Ordered by when phase C (sq-outer, h-inner) needs them."""
            qT, kvT = st["qT"], st["kvT"]
            for sl in (1, 2, 3):
                yield from proj(kvT, lambda dt_: dt_ * 128, wkv_sb,
                                bkv_sb[:], [sl])
                v1_make(st, range(4 * sl, 4 * sl + 4))
            kth = kthp.tile([128, S], BF16, tag="kth", name=f"kth_{nc.next_id()}")
            st["kth"] = kth
            nc.sync.dma_start(out=kth[64:128, :], in_=kvT[0:64, :])
            yield from proj(qT[1], lambda dt_: dt_ * 256 + 128, wq_sb,
                            bq_sb[:, 1:2], [0, 1])
            yield from proj(qT[0], lambda dt_: dt_ * 256, wq_sb,
                            bq_sb[:, 0:1], [2, 3])
            yield from proj(qT[1], lambda dt_: dt_ * 256 + 128, wq_sb,
                            bq_sb[:, 1:2], [2, 3])

        def phase_a1(st):
            for sl in range(NSL):
                hst_dma(1, sl * 512, (sl + 1) * 512)
            yield
            alloc_ab(1, st)
            qT, kvT = st["qT"], st["kvT"]
            for sl in range(NSL):
                yield from proj(kvT, lambda dt_: dt_ * 128, wkv_sb,
                                bkv_sb[:], [sl])
                v1_make(st, range(4 * sl, 4 * sl + 4))
            kth = kthp.tile([128, S], BF16, tag="kth", name=f"kth_{nc.next_id()}")
            st["kth"] = kth
            nc.sync.dma_start(out=kth[64:128, :], in_=kvT[0:64, :])
            yield from proj(qT[1], lambda dt_: dt_ * 256 + 128, wq_sb,
                            bq_sb[:, 1:2], [0, 1])
            yield from proj(qT[0], lambda dt_: dt_ * 256, wq_sb,
                            bq_sb[:, 0:1], [0, 1])
            yield from proj(qT[1], lambda dt_: dt_ * 256 + 128, wq_sb,
                            bq_sb[:, 1:2], [2, 3])
            yield from proj(qT[0], lambda dt_: dt_ * 256, wq_sb,
                            bq_sb[:, 0:1], [2, 3])

        def run_gen(g, n):
            for _ in range(n):
                if g is None:
                    return None
                try:
                    next(g)
                except StopIteration:
                    return None
            return g

        # ---- Phase C (per batch): flat pipelined schedule over
        # g = (iter, kt) with iter = (sq, h).
        def phase_c(b, st, interleave, rate, horder=(0, 1, 2, 3)):
            # kth/v1 are created lazily by the interleaved phase-A generator,
            # always before the emission step that references them.
            qT, kvT, v1 = st["qT"], st["kvT"], st["v1"]
            iters = [(sq, h) for sq in range(NSQ) for h in horder]
            heads_done = [0] * NSQ
            nstep = len(iters) * NKT
            outt = [outp.tile([128, MCOLS], F32, tag="out",
                              name=f"outt{b}_{i}") for i in range(16)]
            prev = None   # (ex_tile, kt, ctx_tile)
            cur_ctx = None

            def scores(g):
                nonlocal cur_ctx
                it, kt = divmod(g, NKT)
                sq, h = iters[it]
                qrow = (h % 2) * 64
                kmat = kvT if qrow == 0 else st["kth"]
                qt = qT[h // 2]
                pss = pssp.tile([128, 1024], F32, tag="pss", name=f"pss_{nc.next_id()}")
                q0 = sq * 1024
                # Boost priority: the scheduler must never park the next
                # scores behind a backlog of ready PV matmuls — the exp
                # stream (the roofline) feeds off scores.
                with tc.high_priority(offset=64):
                    for qc in range(2):
                        nc.tensor.matmul(
                            pss[:, qc * 512:(qc + 1) * 512],
                            kmat[qrow:qrow + 64, kt * 128:(kt + 1) * 128],
                            qt[qrow:qrow + 64,
                               q0 + qc * 512:q0 + (qc + 1) * 512],
                            start=True, stop=True,
                        )
                if kt == 0:
                    cur_ctx = ctxp.tile([128, 1024], F32, tag="ctx", name=f"ctx_{nc.next_id()}")
                return pss

            def expi(pss):
                ex = expp.tile([128, 1024], BF16, tag="exp", name=f"ex_{nc.next_id()}")
                nc.scalar.activation(ex[:], pss[:], Exp, bias=zb[:])
                return ex

            def pv(state):
                # start marks the whole 2KB psum bank pending-zero, so only
                # the first block per bank starts the group; the other
                # blocks' first writes land on pending bytes and overwrite.
                ex, kt, ctx_t = state
                for blk in range(8):
                    nc.tensor.matmul(
                        ctx_t[:, blk * 128:blk * 128 + 65],
                        ex[:, blk * 128:(blk + 1) * 128],
                        v1[kt][:],
                        start=(kt == 0 and blk % 4 == 0),
                        stop=(kt == NKT - 1 and blk % 4 == 3),
                    )

            def norm(it, ctx_t):
                sq, h = iters[it]
                heads_done[sq] += 1
                flush = heads_done[sq] == QH
                for blk in range(8):
                    st_i = sq * 8 + blk
                    rec = recp.tile([128, 1], F32, tag="rec",
                                    name=f"rec_{nc.next_id()}")
                    nc.vector.reciprocal(
                        rec[:], ctx_t[:, blk * 128 + 64:blk * 128 + 65])
                    nc.vector.tensor_scalar_mul(
                        outt[st_i][:, h * 64:(h + 1) * 64],
                        ctx_t[:, blk * 128:blk * 128 + 64], rec[:],
                    )
                    if flush:
                        nc.sync.dma_start(
                            out=out_d[b, st_i * 128:(st_i + 1) * 128, :],
                            in_=outt[st_i][:],
                        )

            for g in range(nstep):
                pss = scores(g)
                ex = expi(pss)
                # interleave right after scores/exp: the PV and next scores
                # stay adjacent in PE order at iteration boundaries.
                interleave = run_gen(interleave, rate(g))
                if prev is not None:
                    pv(prev)
                    if prev[1] == NKT - 1:
                        norm(g // NKT - 1, prev[2])
                prev = (ex, g % NKT, cur_ctx)
            pv(prev)
            norm(len(iters) - 1, prev[2])
            return interleave

        def chain(*gens):
            for g in gens:
                yield from g

        st0, st1 = {}, {}
        alloc_ab(0, st0)
        # Eager startup: kv slice 0, q0 slices 0-1, then v1 tiles (v1 last:
        # its psum pool slots would otherwise gate q0's psum on DVE copies).
        for _ in proj(st0["kvT"], lambda dt_: dt_ * 128, wkv_sb,
                      bkv_sb[:], [0]):
            pass
        for _ in proj(st0["qT"][0], lambda dt_: dt_ * 256, wq_sb,
                      bq_sb[:, 0:1], [0, 1]):
            pass
        v1_make(st0, range(4))

        rest = chain(phase_a_rest(0, st0), phase_a1(st1))
        # Interleave pacing: kv slice sl must be fully EMITTED before
        # scores(kt=4*sl) (emission order is dependency order), but not so
        # early that the PE stalls on its hs^T slice still in flight.
        def rate0(g):
            if g < 11:
                return 5
            return (2, 2, 1)[g % 3]
        rest = phase_c(0, st0, rest, rate0)
        rest = phase_c(1, st1, rest, lambda g: 2, horder=(2, 3, 0, 1))
        while rest is not None:          # safety net; normally exhausted
            rest = run_gen(rest, 16)

    nc.compile()
    return nc


def make_in_maps(hidden_states, Wq, bq, Wk, bk, Wv, bv):
    hs = np.asarray(hidden_states, dtype=np.float32)
    hst = np.ascontiguousarray(hs.transpose(0, 2, 1)).astype(BF16_NP)
    Wq = np.asarray(Wq, dtype=np.float32)
    bq = np.asarray(bq, dtype=np.float32)
    Wk = np.asarray(Wk, dtype=np.float32)
    bk = np.asarray(bk, dtype=np.float32)
    Wv = np.asarray(Wv, dtype=np.float32)
    bv = np.asarray(bv, dtype=np.float32)
    sc = 1.0 / np.sqrt(np.float32(HD))
    ident = np.eye(128, dtype=np.float32).astype(BF16_NP)

    def tile_weights(w):
        # [D, C] -> [128, NDT*C] with layout [p, t*C + c] = w[t*128+p, c]
        cdim = w.shape[1]
        return np.ascontiguousarray(
            w.reshape(NDT, 128, cdim).transpose(1, 0, 2).reshape(128, -1)
        ).astype(BF16_NP)

    in_maps = []
    for c in range(NCORES):
        qs = slice(c * MCOLS, (c + 1) * MCOLS)
        ks = slice(c * HD, (c + 1) * HD)
        bq_c = (bq[qs] * sc).reshape(2, 128).T
        in_maps.append({
            "hst": hst,
            "wq": tile_weights(Wq[:, qs] * sc),
            "wkv": tile_weights(
                np.concatenate([Wk[:, ks], Wv[:, ks]], axis=1)),
            "bq": np.ascontiguousarray(bq_c),
            "bkv": np.concatenate([bk[ks], bv[ks]]).reshape(128, 1),
            "ident": ident,
        })
    return in_maps


_NC_CACHE = {}


def get_nc():
    if "nc" not in _NC_CACHE:
        _NC_CACHE["nc"] = build_nc()
    return _NC_CACHE["nc"]


def kernel(hidden_states, Wq, bq, Wk, bk, Wv, bv):
    nc = get_nc()
    in_maps = make_in_maps(hidden_states, Wq, bq, Wk, bk, Wv, bv)
    res = run_bass_kernel_spmd(nc, in_maps, list(range(NCORES)))
    outs = [np.asarray(r["out"], dtype=np.float32) for r in res.results]
    return np.concatenate(outs, axis=-1)
